# revision 13
# baseline (speedup 1.0000x reference)
"""Full-width attention (B=4, S=2048, D=1024, no head split) on 8 TRN2 cores.

Sharding: data-parallel over (batch, query-half) -> 8 shards. Core c handles
batch b = c//2, query rows [h*1024, (h+1)*1024) with h = c%2.

Algorithm (FLOP-minimal per core: 12.9 GF = 768 N=512 matmuls, bf16):
  - Fold Wq/Wk into the QUERY side: Q'' = (x_q M + w3) / 8 with M = Wq^T Wk
    and w3 = Wk^T bq. scores^T[k,q] = sum_e x[k,e] Q''[q,e] needs no key
    projection. Softmax without max-subtraction (|scores| <= ~25).
  - out[q,e] = (x^T E)^T Wv^T scaled by 1/rowsum + bv.
  - All big matmuls bf16 x bf16 (216 ns/MM at N=512, warm): PE streams
    gap-free at ~99% occupancy for the whole compute window.

Round-2 changes vs the 187.0us version (trace-driven; now ~184.3-186.3
depending on DMA-arrival jitter, in the 2.4GHz chip state):
  - Warmup = 9 MMs, tuned so the ones-warmup ends exactly at the first
    m-chunk arrival (~11.9us, HBM-contention-pinned across the 8 cores)
    AND pre-burns the full ~3.4us HAM window, so ko0 runs warm at 216
    ns/MM instead of cold at 427. After that the 768-MM stream is
    gap-free to the end (verified <0.5us total stalls).
  - The xt-tail/wv/xn DMA issues moved off the Scalar queue: each issue
    costs 600-800ns of queue-engine time and was serializing with the
    qc0 PSUM evacuations (ACTIVATE), stalling qc1's first MMs ~1us.
  - qc0/qc1 evacuations alternate Scalar (activation+bias) and DVE
    (tensor_scalar_add): evac rate 432 -> ~216 ns/bank matches the MM
    consume rate at the sweep boundary.
  - Phase 4 last group de-interleaved (b fully, then c split in 2x256)
    so the post-last-MM tail is one small fuse + 64KB DMA instead of
    two serialized 512-col fuses + 128KB DMAs.
Measured invariants (don't re-litigate): per-NC HBM ~320-360GB/s total,
~160/ring steady, first-chunk completion ~11.9us (ring-arm + contention);
a 3rd DMA path (gpsimd SWDGE) causes 3-way contention, stalls ko1-3 by
~5us and re-throttles HAM -- strictly worse. Phase-1's end is pinned by
TOTAL 4MB arrival (~25.8us), so no phase-1 restructure (finer chunks,
eo/qc generations) can beat the current ko-outer sweep. fp8/DoubleRow is
precision-infeasible everywhere (needs <=2e-2 l2; e4m3 alone adds ~3.6%).
Periodic +163ns PE hiccups every 10.79us (~2.3us total) are firmware.
Run-to-run: chip sometimes drops to 2.0GHz (P0 power state) -> all MMs
259ns and ~+35us total; not code-dependent, ignore those samples.
"""

import math
from contextlib import ExitStack

import numpy as np

P = 128
B, S, D = 4, 2048, 1024
SQ = 1024  # query rows per core
KO8 = 8  # 1024 contraction / 128
KO16 = 16  # 2048 contraction / 128
N_CORES = 8
N_WARM = 8


def build_bass():
    from concourse import bacc
    import concourse.mybir as mybir
    from concourse.tile import TileContext

    f32 = mybir.dt.float32
    f32r = mybir.dt.float32r
    bf16 = mybir.dt.bfloat16
    AF = mybir.ActivationFunctionType

    nc = bacc.Bacc(
        "TRN2",
        target_bir_lowering=False,
        debug=False,
        enable_asserts=False,
        num_devices=N_CORES,
    )

    xT = nc.dram_tensor("xT", [D, S], bf16, kind="ExternalInput")
    xn = nc.dram_tensor("xn", [S, D], bf16, kind="ExternalInput")
    mT = nc.dram_tensor("mT", [D, D], bf16, kind="ExternalInput")
    wvT = nc.dram_tensor("wvT", [D, D], bf16, kind="ExternalInput")
    w3 = nc.dram_tensor("w3", [P, KO8], f32, kind="ExternalInput")
    bvb = nc.dram_tensor("bvb", [P, D], f32, kind="ExternalInput")
    # bf16 output halves the out-DMA traffic (the tail's exposed transfer);
    # host converts back to f32 — quantization ~0.2%, well inside margin
    out = nc.dram_tensor("out", [SQ, D], bf16, kind="ExternalOutput")

    xT_r = xT[:, :].rearrange("(ko p) s -> p ko s", p=P)
    xn_r = xn[:, :].rearrange("(ko p) d -> p ko d", p=P)
    mT_r = mT[:, :].rearrange("(ko p) e -> p ko e", p=P)
    wvT_r = wvT[:, :].rearrange("(ko p) e -> p ko e", p=P)

    with TileContext(nc) as tc, ExitStack() as ctx:
        cst_p = ctx.enter_context(tc.tile_pool(name="cst", bufs=1))
        big_p = ctx.enter_context(tc.tile_pool(name="big", bufs=1))
        out_p = ctx.enter_context(tc.tile_pool(name="osp", bufs=3))
        psA_p = ctx.enter_context(tc.tile_pool(name="psA", bufs=3, space="PSUM"))
        psB_p = ctx.enter_context(tc.tile_pool(name="psB", bufs=2, space="PSUM"))
        psC_p = ctx.enter_context(tc.tile_pool(name="psC", bufs=2, space="PSUM"))
        psR_p = ctx.enter_context(tc.tile_pool(name="psR", bufs=1, space="PSUM"))
        dram_p = ctx.enter_context(tc.tile_pool(name="drp", bufs=1, space="DRAM"))

        # warmup operand comes from a memset, not a DMA (walrus rejects
        # memset on f32r tiles, so memset f32 and bitcast for the PE)
        ones_f = cst_p.tile([P, 512], f32, tag="ones", name="ones_f")
        nc.vector.memset(ones_f[:], 1.0)
        ones_t = ones_f[:, :].bitcast(f32r)
        w3_t = cst_p.tile([P, KO8], f32, tag="w3", name="w3_t")
        bvb_t = cst_p.tile([P, D], f32, tag="bvb", name="bvb_t")

        # big residents
        xt_sb = big_p.tile([P, KO8, S], bf16, tag="xt", name="xt_sb")
        xn_sb = big_p.tile([P, KO16, D], bf16, tag="xn", name="xn_sb")
        m_sb = big_p.tile([P, KO8, D], bf16, tag="m", name="m_sb")
        wv_sb = big_p.tile([P, KO8, D], bf16, tag="wv", name="wv_sb")
        qt_sb = big_p.tile([P, KO8, SQ], bf16, tag="qt", name="qt_sb")
        e_sb = [
            big_p.tile([P, KO16, 512], bf16, tag=f"E{qc}", name=f"e_sb{qc}")
            for qc in range(2)
        ]
        px_sb = big_p.tile([P, KO8, SQ], bf16, tag="px", name="px_sb")
        racc = [
            cst_p.tile([P, 512], f32r, tag=f"racc{qc}", name=f"racc{qc}")
            for qc in range(2)
        ]
        rs_dram = dram_p.tile([1, SQ], f32, tag="rsd", name="rs_dram")

        # Phase-1 feed: per-ko full-width chunks, m on the sync ring and x^T
        # query columns on the scalar ring in parallel. Chunk ko=0 of each is
        # split in half so the first MM group gates on a 128KB transfer.
        # All non-phase-1 loads ride the SYNC ring behind the m chunks: their
        # ~700ns issue slots must not serialize with the qc0 evacuations on
        # the Scalar engine queue, and their transfers stay behind the
        # critical chunks in ring-FIFO order.
        # Phase 1 runs as 4 passes of (eo-half x qc-half), so each pass's
        # per-ko critical bytes are 1KB/partition/ring: pass A (eo0-3,qc0)
        # needs only m cols 0:512 + xt cols 0:512 per ko, and its 4-MM/ko
        # consumption rate (~0.86us) matches the contended per-ring arrival
        # rate (~0.8us/128KB). The m eo-halves ride sync, the xt qc-halves
        # ride scalar, so the two rings deliver the halves passes need in
        # exactly the order they're consumed. (Measured: phase-1 was
        # arrival-bound end-to-end with full 2KB chunks; SWDGE as a 3rd
        # path is a net loss — per-NC HBM caps ~320-360 GB/s total.)
        for ko in range(KO8):
            nc.sync.dma_start(m_sb[:, ko, 0:512], mT_r[:, ko, 0:512])
            nc.scalar.dma_start(xt_sb[:, ko, 0:512], xT_r[:, ko, 0:512])
        # w3 on sync (tiny; needed by the first evacuations ~18us in)
        nc.sync.dma_start(w3_t[:], w3[:, :])
        # the m eo4-7 halves come as ONE dma: pass B runs third (after C),
        # so this only needs to land by ~25us — one issue avoids the ~0.4us
        # per-dma completion overhead that stalled an 8-dma tail
        nc.sync.dma_start(m_sb[:, :, 512:D], mT_r[:, :, 512:D])
        for ko in range(KO8):
            nc.scalar.dma_start(xt_sb[:, ko, 512:SQ], xT_r[:, ko, 512:SQ])
        nc.scalar.dma_start(bvb_t[:], bvb[:, :])
        # non-critical bulk loads: sync ring, behind the m halves
        for kp in range(2):
            nc.sync.dma_start(
                xt_sb[:, 4 * kp : 4 * kp + 4, SQ:S],
                xT_r[:, 4 * kp : 4 * kp + 4, SQ:S],
            )
        nc.sync.dma_start(wv_sb[:, :, :], wvT_r[:, :, :])
        for kp in range(2):
            nc.sync.dma_start(
                xn_sb[:, 8 * kp : 8 * kp + 8, :], xn_r[:, 8 * kp : 8 * kp + 8, :]
            )

        # Short PE warm-up on the ones tile: covers the gap until the first
        # half-chunks land (~9us); the remaining HAM-cold window is spent on
        # real MMs (the HAM only counts busy-time, not usefulness).
        warm = psR_p.tile([1, 512], f32, tag="psR", name="warm")
        for _ in range(N_WARM):
            nc.tensor.matmul(warm[:], ones_t[:, 0:1], ones_t[:, :])

        # ---- Phase 1: Q''T[e, q] = M^T x_q^T + w3 (scaled by 1/8 on host) --
        # 4 passes of (eo-half, qc-half), each ko-OUTER over 4 open PSUM
        # banks, pacing with the half-chunk DMA arrivals (see DMA comment).
        # Pass order keeps the xt qc1-halves (latest arrivals) for last.
        bank_pools = [
            [(psA_p, "psA"), (psA_p, "psA"), (psA_p, "psA"), (psB_p, "psB")],
            [(psB_p, "psB"), (psC_p, "psC"), (psC_p, "psC"), (psR_p, "psR")],
        ]

        def q_pass(h, qc, pi):
            banks = [
                pool.tile([P, 512], f32, tag=tag, name=f"qp{qc}h{h}b{i}")
                for i, (pool, tag) in enumerate(bank_pools[pi])
            ]
            for ko in range(KO8):
                for i in range(4):
                    eo = 4 * h + i
                    nc.tensor.matmul(
                        banks[i][:],
                        m_sb[:, ko, eo * P : (eo + 1) * P],
                        xt_sb[:, ko, qc * 512 : (qc + 1) * 512],
                        start=(ko == 0), stop=(ko == KO8 - 1),
                    )
            # evacuations alternate Scalar/DVE so the next pass's first MMs
            # (which reuse these banks) aren't rate-limited by one engine
            for i in range(4):
                eo = 4 * h + i
                dst = qt_sb[:, eo, qc * 512 : (qc + 1) * 512]
                if i % 2 == 0:
                    nc.scalar.activation(
                        dst, banks[i][:], AF.Identity, bias=w3_t[:, eo : eo + 1]
                    )
                else:
                    nc.vector.tensor_scalar_add(
                        dst, banks[i][:], w3_t[:, eo : eo + 1]
                    )

        # A (eo0-3,qc0) paces with the contended early arrivals; C (eo0-3,
        # qc1) consumes the xt-b stream next; B and D run once the single
        # m-b transfer has landed and stream at full rate
        q_pass(0, 0, 0)
        q_pass(0, 1, 1)
        q_pass(1, 0, 0)
        q_pass(1, 1, 1)

        # ---- Phase 2: scores^T -> exp -> E (bf16), rowsum acc on DVE ------
        for kidx in range(KO16):
            pa = psA_p.tile([P, 512], f32, tag="psA", name="spa")
            pb = psB_p.tile([P, 512], f32, tag="psB", name="spb")
            for eo in range(KO8):
                lh = xt_sb[:, eo, kidx * P : (kidx + 1) * P]
                nc.tensor.matmul(
                    pa[:], lh, qt_sb[:, eo, 0:512],
                    start=(eo == 0), stop=(eo == KO8 - 1),
                )
                nc.tensor.matmul(
                    pb[:], lh, qt_sb[:, eo, 512:1024],
                    start=(eo == 0), stop=(eo == KO8 - 1),
                )
            nc.scalar.activation(e_sb[0][:, kidx, :], pa[:], AF.Exp)
            nc.scalar.activation(e_sb[1][:, kidx, :], pb[:], AF.Exp)
            for qc in range(2):
                if kidx == 0:
                    nc.vector.tensor_copy(racc[qc][:], e_sb[qc][:, 0, :])
                else:
                    nc.vector.tensor_add(
                        racc[qc][:], racc[qc][:], e_sb[qc][:, kidx, :]
                    )

        # ---- Phase 3: PX^T[d, q] = sum_k x[k, d] E[k, q] -------------------
        for dc in range(KO8):
            pp = psA_p.tile([P, 512], f32, tag="psA", name="ppx")
            for ko in range(KO16):
                nc.tensor.matmul(
                    pp[:],
                    xn_sb[:, ko, dc * P : (dc + 1) * P],
                    e_sb[0][:, ko, :],
                    start=(ko == 0), stop=(ko == KO16 - 1),
                )
            nc.scalar.copy(px_sb[:, dc, 0:512], pp[:])

        # rowsum partition-reduce + [1,1024] -> [128,8] recip via DRAM bounce
        # (PE cost ~2 tiny matmuls; bounce hides under PX)
        for qc in range(2):
            pr = psR_p.tile([1, 512], f32, tag="psR", name="pr")
            nc.tensor.matmul(pr[:], ones_t[:, 0:1], racc[qc][:])
            rrow = cst_p.tile([1, 512], f32, tag=f"rr{qc}", name=f"rrow{qc}")
            nc.scalar.copy(rrow[:], pr[:])
            nc.sync.dma_start(rs_dram[0:1, qc * 512 : (qc + 1) * 512], rrow[:])
        rsum_t = cst_p.tile([P, 8], f32, tag="rst", name="rsum_t")
        nc.sync.dma_start(rsum_t[:, :], rs_dram[0, :].rearrange("(g p) -> p g", p=P))
        recip = cst_p.tile([P, 8], f32, tag="recip", name="recip")
        nc.vector.reciprocal(recip[:], rsum_t[:])

        for dc in range(KO8):
            pp = psA_p.tile([P, 512], f32, tag="psA", name="ppx")
            for ko in range(KO16):
                nc.tensor.matmul(
                    pp[:],
                    xn_sb[:, ko, dc * P : (dc + 1) * P],
                    e_sb[1][:, ko, :],
                    start=(ko == 0), stop=(ko == KO16 - 1),
                )
            nc.scalar.copy(px_sb[:, dc, 512:1024], pp[:])

        # ---- Phase 4: out[q, e] = PX^T.T Wv^T / rowsum + bv ---------------
        mul, add = mybir.AluOpType.mult, mybir.AluOpType.add

        def av_fuse(ps, g, c0, c1):
            # fused (psum * recip) + bv straight from PSUM on DVE, then DMA
            # (Pool/gpsimd cannot read PSUM on TRN2)
            o = out_p.tile([P, 512], bf16, tag="ost", name="ost")
            nc.vector.scalar_tensor_tensor(
                o[:, 0 : c1 - c0], ps[:], recip[:, g : g + 1],
                bvb_t[:, c0:c1], mul, add,
            )
            nc.sync.dma_start(out[g * P : (g + 1) * P, c0:c1], o[:, 0 : c1 - c0])

        for g in range(7):
            pb = psB_p.tile([P, 512], f32, tag="psB", name="avb")
            pc = psC_p.tile([P, 512], f32, tag="psC", name="avc")
            for dc in range(KO8):
                lh = px_sb[:, dc, g * P : (g + 1) * P]
                nc.tensor.matmul(
                    pb[:], lh, wv_sb[:, dc, 0:512],
                    start=(dc == 0), stop=(dc == KO8 - 1),
                )
                nc.tensor.matmul(
                    pc[:], lh, wv_sb[:, dc, 512:1024],
                    start=(dc == 0), stop=(dc == KO8 - 1),
                )
            av_fuse(pb, g, 0, 512)
            av_fuse(pc, g, 512, 1024)

        # last group de-interleaved: b completes (fuse+DMA overlap c's MMs),
        # then c in two 256-col accumulations so the post-last-MM tail is a
        # single small fuse + 64KB DMA.
        g = 7
        pb = psB_p.tile([P, 512], f32, tag="psB", name="avb")
        lhs = [px_sb[:, dc, g * P : (g + 1) * P] for dc in range(KO8)]
        for dc in range(KO8):
            nc.tensor.matmul(
                pb[:], lhs[dc], wv_sb[:, dc, 0:512],
                start=(dc == 0), stop=(dc == KO8 - 1),
            )
        av_fuse(pb, g, 0, 512)
        for half, c0 in ((0, 512), (1, 768)):
            pc = psC_p.tile([P, 256], f32, tag="psC", name=f"avc{half}")
            for dc in range(KO8):
                nc.tensor.matmul(
                    pc[:], lhs[dc], wv_sb[:, dc, c0 : c0 + 256],
                    start=(dc == 0), stop=(dc == KO8 - 1),
                )
            av_fuse(pc, g, c0, c0 + 256)

    nc.finalize()
    return nc


def make_in_maps(x, Wq, bq, Wk, bk, Wv, bv):
    """Build the 8 per-core input maps from full inputs."""
    import ml_dtypes

    bf = ml_dtypes.bfloat16
    x = np.asarray(x, dtype=np.float32)
    inv8 = 1.0 / math.sqrt(D // 16)  # 1/sqrt(d_key=64) = 1/8
    # scores = x_q (Wq^T Wk) x_k^T / 8 + x_k.(Wk^T bq)/8 (+ softmax-invariant
    # per-query terms, dropped). Both folded into the query-side projection.
    M8 = (
        (np.asarray(Wq, np.float64).T @ np.asarray(Wk, np.float64)) * inv8
    ).astype(bf)
    w3 = (
        (np.asarray(Wk, np.float64).T @ np.asarray(bq, np.float64)) * inv8
    ).astype(np.float32)
    w3_np = np.ascontiguousarray(w3.reshape(KO8, P).T)
    wvT = np.ascontiguousarray(np.asarray(Wv, np.float32).T.astype(bf))
    bvb = np.ascontiguousarray(
        np.broadcast_to(np.asarray(bv, np.float32), (P, D))
    )
    in_maps = []
    for c in range(N_CORES):
        b, h = c // 2, c % 2
        # rotate the key axis by h*SQ so this core's queries are always
        # columns 0:SQ of xT; attention is permutation-invariant over keys
        # as long as xT (scores lhsT) and xn (PX lhsT) rotate together.
        xb = np.roll(x[b], -h * SQ, axis=0)
        in_maps.append(
            {
                "xT": np.ascontiguousarray(xb.T.astype(bf)),
                "xn": np.ascontiguousarray(xb.astype(bf)),
                "mT": M8,
                "wvT": wvT,
                "w3": w3_np,
                "bvb": bvb,
            }
        )
    return in_maps


_NC_CACHE = None


def get_nc():
    global _NC_CACHE
    if _NC_CACHE is None:
        _NC_CACHE = build_bass()
    return _NC_CACHE


def kernel(x, Wq, bq, Wk, bk, Wv, bv, **run_kwargs):
    from concourse.bass_utils import run_bass_kernel_spmd

    nc = get_nc()
    in_maps = make_in_maps(x, Wq, bq, Wk, bk, Wv, bv)
    res = run_bass_kernel_spmd(
        nc, in_maps, core_ids=list(range(N_CORES)), **run_kwargs
    )
    out = np.empty((B, S, D), dtype=np.float32)
    for c in range(N_CORES):
        b, h = c // 2, c % 2
        out[b, h * SQ : (h + 1) * SQ, :] = np.asarray(
            res.results[c]["out"], dtype=np.float32
        )
    if run_kwargs.get("trace"):
        kernel.last_results = res
    return out


# revision 15
# speedup vs baseline: 1.0200x; 1.0200x over previous
"""Full-width attention (B=4, S=2048, D=1024, no head split) on 8 TRN2 cores.

Sharding: data-parallel over (batch, query-half) -> 8 shards. Core c handles
batch b = c//2, query rows [h*1024, (h+1)*1024) with h = c%2.

Algorithm (FLOP-minimal per core: 12.9 GF = 768 N=512 matmuls, bf16):
  - Fold Wq/Wk into the QUERY side: Q'' = (x_q M + w3) / 8 with M = Wq^T Wk
    and w3 = Wk^T bq. scores^T[k,q] = sum_e x[k,e] Q''[q,e] needs no key
    projection. Softmax without max-subtraction (|scores| <= ~25).
  - out[q,e] = (x^T E)^T Wv^T scaled by 1/rowsum + bv.
  - All big matmuls bf16 x bf16 (216 ns/MM at N=512, warm): PE streams
    gap-free at ~99% occupancy for the whole compute window.

Round-2 changes vs the 187.0us version (trace-driven; now ~184.3-186.3
depending on DMA-arrival jitter, in the 2.4GHz chip state):
  - Warmup = 9 MMs, tuned so the ones-warmup ends exactly at the first
    m-chunk arrival (~11.9us, HBM-contention-pinned across the 8 cores)
    AND pre-burns the full ~3.4us HAM window, so ko0 runs warm at 216
    ns/MM instead of cold at 427. After that the 768-MM stream is
    gap-free to the end (verified <0.5us total stalls).
  - The xt-tail/wv/xn DMA issues moved off the Scalar queue: each issue
    costs 600-800ns of queue-engine time and was serializing with the
    qc0 PSUM evacuations (ACTIVATE), stalling qc1's first MMs ~1us.
  - qc0/qc1 evacuations alternate Scalar (activation+bias) and DVE
    (tensor_scalar_add): evac rate 432 -> ~216 ns/bank matches the MM
    consume rate at the sweep boundary.
  - Phase 4 last group de-interleaved (b fully, then c split in 2x256)
    so the post-last-MM tail is one small fuse + 64KB DMA instead of
    two serialized 512-col fuses + 128KB DMAs.
Measured invariants (don't re-litigate): per-NC HBM ~320-360GB/s total,
~160/ring steady, first-chunk completion ~11.9us (ring-arm + contention);
a 3rd DMA path (gpsimd SWDGE) causes 3-way contention, stalls ko1-3 by
~5us and re-throttles HAM -- strictly worse. Phase-1's end is pinned by
TOTAL 4MB arrival (~25.8us), so no phase-1 restructure (finer chunks,
eo/qc generations) can beat the current ko-outer sweep. fp8/DoubleRow is
precision-infeasible everywhere (needs <=2e-2 l2; e4m3 alone adds ~3.6%).
Periodic +163ns PE hiccups every 10.79us (~2.3us total) are firmware.
Run-to-run: chip sometimes drops to 2.0GHz (P0 power state) -> all MMs
259ns and ~+35us total; not code-dependent, ignore those samples.
"""

import math
from contextlib import ExitStack

import numpy as np

P = 128
B, S, D = 4, 2048, 1024
SQ = 1024  # query rows per core
KO8 = 8  # 1024 contraction / 128
KO16 = 16  # 2048 contraction / 128
N_CORES = 8
N_WARM = 9


def build_bass():
    from concourse import bacc
    import concourse.mybir as mybir
    from concourse.tile import TileContext

    f32 = mybir.dt.float32
    f32r = mybir.dt.float32r
    bf16 = mybir.dt.bfloat16
    AF = mybir.ActivationFunctionType

    nc = bacc.Bacc(
        "TRN2",
        target_bir_lowering=False,
        debug=False,
        enable_asserts=False,
        num_devices=N_CORES,
    )

    xT = nc.dram_tensor("xT", [D, S], bf16, kind="ExternalInput")
    xn = nc.dram_tensor("xn", [S, D], bf16, kind="ExternalInput")
    mT = nc.dram_tensor("mT", [D, D], bf16, kind="ExternalInput")
    wvT = nc.dram_tensor("wvT", [D, D], bf16, kind="ExternalInput")
    w3 = nc.dram_tensor("w3", [P, KO8], f32, kind="ExternalInput")
    bvb = nc.dram_tensor("bvb", [P, D], f32, kind="ExternalInput")
    # bf16 output halves the out-DMA traffic (the tail's exposed transfer);
    # host converts back to f32 — quantization ~0.2%, well inside margin
    out = nc.dram_tensor("out", [SQ, D], bf16, kind="ExternalOutput")

    xT_r = xT[:, :].rearrange("(ko p) s -> p ko s", p=P)
    xn_r = xn[:, :].rearrange("(ko p) d -> p ko d", p=P)
    mT_r = mT[:, :].rearrange("(ko p) e -> p ko e", p=P)
    wvT_r = wvT[:, :].rearrange("(ko p) e -> p ko e", p=P)

    with TileContext(nc) as tc, ExitStack() as ctx:
        cst_p = ctx.enter_context(tc.tile_pool(name="cst", bufs=1))
        big_p = ctx.enter_context(tc.tile_pool(name="big", bufs=1))
        out_p = ctx.enter_context(tc.tile_pool(name="osp", bufs=3))
        psA_p = ctx.enter_context(tc.tile_pool(name="psA", bufs=3, space="PSUM"))
        psB_p = ctx.enter_context(tc.tile_pool(name="psB", bufs=2, space="PSUM"))
        psC_p = ctx.enter_context(tc.tile_pool(name="psC", bufs=2, space="PSUM"))
        psR_p = ctx.enter_context(tc.tile_pool(name="psR", bufs=1, space="PSUM"))
        dram_p = ctx.enter_context(tc.tile_pool(name="drp", bufs=1, space="DRAM"))

        # warmup operand comes from a memset, not a DMA (walrus rejects
        # memset on f32r tiles, so memset f32 and bitcast for the PE)
        ones_f = cst_p.tile([P, 512], f32, tag="ones", name="ones_f")
        nc.vector.memset(ones_f[:], 1.0)
        ones_t = ones_f[:, :].bitcast(f32r)
        w3_t = cst_p.tile([P, KO8], f32, tag="w3", name="w3_t")
        bvb_t = cst_p.tile([P, D], f32, tag="bvb", name="bvb_t")

        # big residents
        xt_sb = big_p.tile([P, KO8, S], bf16, tag="xt", name="xt_sb")
        xn_sb = big_p.tile([P, KO16, D], bf16, tag="xn", name="xn_sb")
        m_sb = big_p.tile([P, KO8, D], bf16, tag="m", name="m_sb")
        wv_sb = big_p.tile([P, KO8, D], bf16, tag="wv", name="wv_sb")
        qt_sb = big_p.tile([P, KO8, SQ], bf16, tag="qt", name="qt_sb")
        e_sb = [
            big_p.tile([P, KO16, 512], bf16, tag=f"E{qc}", name=f"e_sb{qc}")
            for qc in range(2)
        ]
        px_sb = big_p.tile([P, KO8, SQ], bf16, tag="px", name="px_sb")
        racc = [
            cst_p.tile([P, 512], f32r, tag=f"racc{qc}", name=f"racc{qc}")
            for qc in range(2)
        ]
        rs_dram = dram_p.tile([1, SQ], f32, tag="rsd", name="rs_dram")

        # Phase-1 feed: per-ko full-width chunks (one dma each — finer splits
        # pay ~0.4us/dma completion overhead), m on the sync ring and x^T
        # query columns on the scalar ring in parallel. All non-phase-1
        # loads ride the SYNC ring behind the m chunks: their ~700ns issue
        # slots must not serialize with the qc0 evacuations on the Scalar
        # engine queue, and their transfers stay behind the critical chunks
        # in ring-FIFO order. (SWDGE as a 3rd path is a net loss — per-NC
        # HBM caps ~320-360 GB/s total.)
        for ko in range(KO8):
            nc.sync.dma_start(m_sb[:, ko, :], mT_r[:, ko, :])
            nc.scalar.dma_start(xt_sb[:, ko, 0:SQ], xT_r[:, ko, 0:SQ])
        # small consts on the scalar ring (w3 first used ~22us in)
        nc.scalar.dma_start(w3_t[:], w3[:, :])
        nc.scalar.dma_start(bvb_t[:], bvb[:, :])
        # non-critical bulk loads: sync ring, behind the m halves
        for kp in range(2):
            nc.sync.dma_start(
                xt_sb[:, 4 * kp : 4 * kp + 4, SQ:S],
                xT_r[:, 4 * kp : 4 * kp + 4, SQ:S],
            )
        nc.sync.dma_start(wv_sb[:, :, :], wvT_r[:, :, :])
        for kp in range(2):
            nc.sync.dma_start(
                xn_sb[:, 8 * kp : 8 * kp + 8, :], xn_r[:, 8 * kp : 8 * kp + 8, :]
            )

        # Short PE warm-up on the ones tile: covers the gap until the first
        # half-chunks land (~9us); the remaining HAM-cold window is spent on
        # real MMs (the HAM only counts busy-time, not usefulness).
        warm = psR_p.tile([1, 512], f32, tag="psR", name="warm")
        for _ in range(N_WARM):
            nc.tensor.matmul(warm[:], ones_t[:, 0:1], ones_t[:, :])

        # ---- Phase 1: Q''T[e, q] = M^T x_q^T + w3 (scaled by 1/8 on host) --
        # ko-OUTER with all 8 eo accumulations held open across the full
        # PSUM bank set: each ko step needs only chunk ko of m/x^T, so the
        # whole sweep paces with the DMA chunk arrivals. (4-pass eo/qc-half
        # restructures with split/merged DMAs were measured at 184.9-187.9
        # vs 184.0-186.3 for this layout: per-dma completion overhead
        # (~0.4us) and cross-ring HBM contention eat the theoretical gain.)
        def q_sweep(qc):
            banks = [
                psA_p.tile([P, 512], f32, tag="psA", name=f"qp{qc}a{i}")
                for i in range(3)
            ] + [
                psB_p.tile([P, 512], f32, tag="psB", name=f"qp{qc}b{i}")
                for i in range(2)
            ] + [
                psC_p.tile([P, 512], f32, tag="psC", name=f"qp{qc}c{i}")
                for i in range(2)
            ] + [psR_p.tile([P, 512], f32, tag="psR", name=f"qp{qc}r")]
            for ko in range(KO8):
                for eo in range(KO8):
                    nc.tensor.matmul(
                        banks[eo][:],
                        m_sb[:, ko, eo * P : (eo + 1) * P],
                        xt_sb[:, ko, qc * 512 : (qc + 1) * 512],
                        start=(ko == 0), stop=(ko == KO8 - 1),
                    )
            # evacuations alternate Scalar/DVE so the next sweep's first MMs
            # (which reuse these banks) aren't rate-limited by one engine
            for eo in range(KO8):
                dst = qt_sb[:, eo, qc * 512 : (qc + 1) * 512]
                if eo % 2 == 0:
                    nc.scalar.activation(
                        dst, banks[eo][:], AF.Identity, bias=w3_t[:, eo : eo + 1]
                    )
                else:
                    nc.vector.tensor_scalar_add(
                        dst, banks[eo][:], w3_t[:, eo : eo + 1]
                    )

        q_sweep(0)
        q_sweep(1)

        # ---- Phase 2: scores^T -> exp -> E (bf16), rowsum acc on DVE ------
        for kidx in range(KO16):
            pa = psA_p.tile([P, 512], f32, tag="psA", name="spa")
            pb = psB_p.tile([P, 512], f32, tag="psB", name="spb")
            for eo in range(KO8):
                lh = xt_sb[:, eo, kidx * P : (kidx + 1) * P]
                nc.tensor.matmul(
                    pa[:], lh, qt_sb[:, eo, 0:512],
                    start=(eo == 0), stop=(eo == KO8 - 1),
                )
                nc.tensor.matmul(
                    pb[:], lh, qt_sb[:, eo, 512:1024],
                    start=(eo == 0), stop=(eo == KO8 - 1),
                )
            nc.scalar.activation(e_sb[0][:, kidx, :], pa[:], AF.Exp)
            nc.scalar.activation(e_sb[1][:, kidx, :], pb[:], AF.Exp)
            for qc in range(2):
                if kidx == 0:
                    nc.vector.tensor_copy(racc[qc][:], e_sb[qc][:, 0, :])
                else:
                    nc.vector.tensor_add(
                        racc[qc][:], racc[qc][:], e_sb[qc][:, kidx, :]
                    )

        # ---- Phase 3: PX^T[d, q] = sum_k x[k, d] E[k, q] -------------------
        for dc in range(KO8):
            pp = psA_p.tile([P, 512], f32, tag="psA", name="ppx")
            for ko in range(KO16):
                nc.tensor.matmul(
                    pp[:],
                    xn_sb[:, ko, dc * P : (dc + 1) * P],
                    e_sb[0][:, ko, :],
                    start=(ko == 0), stop=(ko == KO16 - 1),
                )
            nc.scalar.copy(px_sb[:, dc, 0:512], pp[:])

        # rowsum partition-reduce + [1,1024] -> [128,8] recip via DRAM bounce
        # (PE cost ~2 tiny matmuls; bounce hides under PX)
        for qc in range(2):
            pr = psR_p.tile([1, 512], f32, tag="psR", name="pr")
            nc.tensor.matmul(pr[:], ones_t[:, 0:1], racc[qc][:])
            rrow = cst_p.tile([1, 512], f32, tag=f"rr{qc}", name=f"rrow{qc}")
            nc.scalar.copy(rrow[:], pr[:])
            nc.sync.dma_start(rs_dram[0:1, qc * 512 : (qc + 1) * 512], rrow[:])
        rsum_t = cst_p.tile([P, 8], f32, tag="rst", name="rsum_t")
        nc.sync.dma_start(rsum_t[:, :], rs_dram[0, :].rearrange("(g p) -> p g", p=P))
        recip = cst_p.tile([P, 8], f32, tag="recip", name="recip")
        nc.vector.reciprocal(recip[:], rsum_t[:])

        for dc in range(KO8):
            pp = psA_p.tile([P, 512], f32, tag="psA", name="ppx")
            for ko in range(KO16):
                nc.tensor.matmul(
                    pp[:],
                    xn_sb[:, ko, dc * P : (dc + 1) * P],
                    e_sb[1][:, ko, :],
                    start=(ko == 0), stop=(ko == KO16 - 1),
                )
            nc.scalar.copy(px_sb[:, dc, 512:1024], pp[:])

        # ---- Phase 4: out[q, e] = PX^T.T Wv^T / rowsum + bv ---------------
        mul, add = mybir.AluOpType.mult, mybir.AluOpType.add

        def av_fuse(ps, g, c0, c1):
            # fused (psum * recip) + bv straight from PSUM on DVE, then DMA
            # (Pool/gpsimd cannot read PSUM on TRN2)
            o = out_p.tile([P, 512], bf16, tag="ost", name="ost")
            nc.vector.scalar_tensor_tensor(
                o[:, 0 : c1 - c0], ps[:], recip[:, g : g + 1],
                bvb_t[:, c0:c1], mul, add,
            )
            nc.sync.dma_start(out[g * P : (g + 1) * P, c0:c1], o[:, 0 : c1 - c0])

        for g in range(7):
            pb = psB_p.tile([P, 512], f32, tag="psB", name="avb")
            pc = psC_p.tile([P, 512], f32, tag="psC", name="avc")
            for dc in range(KO8):
                lh = px_sb[:, dc, g * P : (g + 1) * P]
                nc.tensor.matmul(
                    pb[:], lh, wv_sb[:, dc, 0:512],
                    start=(dc == 0), stop=(dc == KO8 - 1),
                )
                nc.tensor.matmul(
                    pc[:], lh, wv_sb[:, dc, 512:1024],
                    start=(dc == 0), stop=(dc == KO8 - 1),
                )
            av_fuse(pb, g, 0, 512)
            av_fuse(pc, g, 512, 1024)

        # last group de-interleaved: b completes (fuse+DMA overlap c's MMs),
        # then c in two 256-col accumulations so the post-last-MM tail is a
        # single small fuse + 64KB DMA.
        g = 7
        pb = psB_p.tile([P, 512], f32, tag="psB", name="avb")
        lhs = [px_sb[:, dc, g * P : (g + 1) * P] for dc in range(KO8)]
        for dc in range(KO8):
            nc.tensor.matmul(
                pb[:], lhs[dc], wv_sb[:, dc, 0:512],
                start=(dc == 0), stop=(dc == KO8 - 1),
            )
        av_fuse(pb, g, 0, 512)
        for half, c0 in ((0, 512), (1, 768)):
            pc = psC_p.tile([P, 256], f32, tag="psC", name=f"avc{half}")
            for dc in range(KO8):
                nc.tensor.matmul(
                    pc[:], lhs[dc], wv_sb[:, dc, c0 : c0 + 256],
                    start=(dc == 0), stop=(dc == KO8 - 1),
                )
            av_fuse(pc, g, c0, c0 + 256)

    nc.finalize()
    return nc


def make_in_maps(x, Wq, bq, Wk, bk, Wv, bv):
    """Build the 8 per-core input maps from full inputs."""
    import ml_dtypes

    bf = ml_dtypes.bfloat16
    x = np.asarray(x, dtype=np.float32)
    inv8 = 1.0 / math.sqrt(D // 16)  # 1/sqrt(d_key=64) = 1/8
    # scores = x_q (Wq^T Wk) x_k^T / 8 + x_k.(Wk^T bq)/8 (+ softmax-invariant
    # per-query terms, dropped). Both folded into the query-side projection.
    M8 = (
        (np.asarray(Wq, np.float64).T @ np.asarray(Wk, np.float64)) * inv8
    ).astype(bf)
    w3 = (
        (np.asarray(Wk, np.float64).T @ np.asarray(bq, np.float64)) * inv8
    ).astype(np.float32)
    w3_np = np.ascontiguousarray(w3.reshape(KO8, P).T)
    wvT = np.ascontiguousarray(np.asarray(Wv, np.float32).T.astype(bf))
    bvb = np.ascontiguousarray(
        np.broadcast_to(np.asarray(bv, np.float32), (P, D))
    )
    in_maps = []
    for c in range(N_CORES):
        b, h = c // 2, c % 2
        # rotate the key axis by h*SQ so this core's queries are always
        # columns 0:SQ of xT; attention is permutation-invariant over keys
        # as long as xT (scores lhsT) and xn (PX lhsT) rotate together.
        xb = np.roll(x[b], -h * SQ, axis=0)
        in_maps.append(
            {
                "xT": np.ascontiguousarray(xb.T.astype(bf)),
                "xn": np.ascontiguousarray(xb.astype(bf)),
                "mT": M8,
                "wvT": wvT,
                "w3": w3_np,
                "bvb": bvb,
            }
        )
    return in_maps


_NC_CACHE = None


def get_nc():
    global _NC_CACHE
    if _NC_CACHE is None:
        _NC_CACHE = build_bass()
    return _NC_CACHE


def kernel(x, Wq, bq, Wk, bk, Wv, bv, **run_kwargs):
    from concourse.bass_utils import run_bass_kernel_spmd

    nc = get_nc()
    in_maps = make_in_maps(x, Wq, bq, Wk, bk, Wv, bv)
    res = run_bass_kernel_spmd(
        nc, in_maps, core_ids=list(range(N_CORES)), **run_kwargs
    )
    out = np.empty((B, S, D), dtype=np.float32)
    for c in range(N_CORES):
        b, h = c // 2, c % 2
        out[b, h * SQ : (h + 1) * SQ, :] = np.asarray(
            res.results[c]["out"], dtype=np.float32
        )
    if run_kwargs.get("trace"):
        kernel.last_results = res
    return out
